# revision 24
# baseline (speedup 1.0000x reference)
"""Trainium2 Bass kernel for nn_ActVQ (VQ-VAE: encoder -> vector-quantize -> decoder).

Contract: kernel(**inputs) takes the FULL inputs from reference.setup_inputs()
(x: (64, 4096, 4) f32 + nested params dict) and returns the FULL output tuple
(y: (64, 4096, 4) f32, commit_loss scalar, perplexity scalar), matching
reference.reference().

Sharding: pure data-parallel over batch N=64 -> 8 NeuronCores x 8 elements.
All parameters are replicated; outputs are gathered/combined on host.

Device layout: activations live in SBUF as (128, T) tiles with partition
p = 16*elem + channel (8 elems x 16 channels).  Convolutions are emitted as
tensor-engine matmuls using 32x32 tile_position packing (4 concurrent diagonal
tiles, each a diag-2 block of the 16x16 channel-mixing matrix); the K taps of a
conv accumulate into PSUM, and PSUM is evicted by the scalar/vector engines
with bias add (+ ReLU / + residual) fused in.

VQ: distances argmin over the 2048-entry codebook is computed exactly over a
candidate subset of K=64 codes chosen on the host by smallest ||c||^2 (the
encoder output x_enc is tiny relative to codebook norms, so only the
smallest-norm codes can win).  This pruning is *validated at runtime* with a
rigorous bound computed from device outputs (max ||x_enc|| and the worst
winning distance); if the bound ever failed (it cannot for the reference input
distribution), the host falls back to an exact argmin over all 2048 codes from
the (also returned) encoder output, and re-runs the decoder on device.
"""

import math
from contextlib import ExitStack

import numpy as np

import concourse.bacc as bacc
import concourse.bass as bass
import concourse.bass_isa as bass_isa
import concourse.mybir as mybir
import concourse.tile as tile
from concourse import bass_utils

F32 = mybir.dt.float32
U32 = mybir.dt.uint32
ALU = mybir.AluOpType
ACTF = mybir.ActivationFunctionType

PAD = 4          # halo columns on each side of activation buffers
K_CAND = 64      # candidate codebook entries (of 2048)
NCODE = 2048
NELEM = 8        # batch elements per core
TFULL = 4096
TENC = 512       # encoder output length per element
CHUNK = 512      # psum free-dim chunk

_NC_CACHE: dict = {}


# --------------------------------------------------------------------------
# static layer program (shared between host weight packer and device builder)
# --------------------------------------------------------------------------

def _cls_of(T):
    return {4096: "A", 2048: "B", 1024: "C", 512: "D"}[T]


def layer_specs():
    """Linear program of ops.  Weight-slot and bias-slot indices are assigned
    in order; the host packer follows the identical order."""
    specs = []
    wslot = [0]
    bslot = [0]

    def conv(kind, ntap, **kw):
        s = dict(kind=kind, w0=wslot[0], nb=bslot[0], **kw)
        wslot[0] += ntap
        bslot[0] += 1
        specs.append(s)

    def res_chain(cls, T, dils):
        xs = [cls + "0", cls + "3", cls + "0", cls + "3"]
        for j, d in enumerate(dils):
            specs.append(dict(kind="relu", T=T, src=xs[j], dst=cls + "1"))
            conv("conv3", 3, T=T, d=d, src=cls + "1", dst=cls + "2", evict="relu")
            conv("conv1", 1, T=T, src=cls + "2", dst=xs[j + 1], evict="res",
                 res=xs[j])
        return xs[3]

    # ---- encoder ----
    conv("conv3", 3, T=4096, d=1, src="A0", dst="A1", evict="relu")
    src = "A1"
    for T_in in (4096, 2048, 1024):
        cls = _cls_of(T_in // 2)
        conv("conv4s2", 4, T=T_in // 2, src=src, dst=cls + "0", evict="plain")
        src = res_chain(cls, T_in // 2, [1, 2, 4])
    conv("conv3", 3, T=512, d=1, src=src, dst="XE", evict="plain")

    specs.append(dict(kind="vq"))

    # ---- decoder ----
    conv("conv3", 3, T=512, d=1, src="XD", dst="D0", evict="relu")
    up_dst = {512: "C0", 1024: "B0", 2048: "A2"}
    src = "D0"
    for T in (512, 1024, 2048):
        cls = _cls_of(T)
        last = res_chain(cls, T, [4, 2, 1])
        conv("upconv", 4, T=2 * T, src=last, dst=up_dst[T], evict="up")
        src = up_dst[T]
    conv("conv3", 3, T=4096, d=1, src="A2", dst="A1", evict="relu")
    conv("conv3", 3, T=4096, d=1, src="A1", dst="A0", evict="plain")

    return specs, wslot[0], bslot[0]


# --------------------------------------------------------------------------
# host-side parameter packing (runtime, numpy)
# --------------------------------------------------------------------------

def _diag2(w16):
    """w16: (out<=16, in<=16) conv-tap matrix -> (32, 32) lhsT block with the
    transposed tap on both 16x16 diagonal blocks (2 batch elems per tile)."""
    m16 = np.zeros((16, 16), np.float32)
    m16[: w16.shape[0], : w16.shape[1]] = w16
    out = np.zeros((32, 32), np.float32)
    out[0:16, 0:16] = m16.T
    out[16:32, 16:32] = m16.T
    return out


def pack_conv_params(params):
    mats, biases = [], []

    def add_conv(w, b):
        w = np.asarray(w, np.float32)
        b = np.asarray(b, np.float32)
        for k in range(w.shape[2]):
            mats.append(_diag2(w[:, :, k]))
        biases.append(np.tile(np.pad(b, (0, 16 - b.shape[0])), NELEM))

    def add_upconv(w, b):
        w = np.asarray(w, np.float32)
        b = np.asarray(b, np.float32)
        for m in (w[:, :, 0], w[:, :, 1] + w[:, :, 2],
                  w[:, :, 0] + w[:, :, 1], w[:, :, 2]):
            mats.append(_diag2(m))
        biases.append(np.tile(np.pad(b, (0, 16 - b.shape[0])), NELEM))

    enc = params["enc"]
    add_conv(enc["in_w"], enc["in_b"])
    for blk in enc["down"]:
        add_conv(blk["w"], blk["b"])
        for rp in blk["res"]:
            add_conv(rp["w1"], rp["b1"])
            add_conv(rp["w2"], rp["b2"])
    add_conv(enc["out_w"], enc["out_b"])

    dec = params["dec"]
    add_conv(dec["in_w"], dec["in_b"])
    for blk in dec["up"]:
        for rp in blk["res"]:
            add_conv(rp["w1"], rp["b1"])
            add_conv(rp["w2"], rp["b2"])
        add_upconv(blk["w"], blk["b"])
    add_conv(dec["mid_w"], dec["mid_b"])
    add_conv(dec["out_w"], dec["out_b"])

    return (np.ascontiguousarray(np.stack(mats)),
            np.ascontiguousarray(np.stack(biases).astype(np.float32).T))


# --------------------------------------------------------------------------
# device kernel builder
# --------------------------------------------------------------------------

def build_nc(NW, NB, decoder_only=False):
    nc = bacc.Bacc("TRN2", target_bir_lowering=False, debug=False)

    specs, nw, nb = layer_specs()
    assert nw == NW and nb == NB

    # ---- DRAM I/O ----
    w_d = nc.dram_tensor("wmats", [NW, 32, 32], F32, kind="ExternalInput")
    b_d = nc.dram_tensor("bvecs", [128, NB], F32, kind="ExternalInput")
    y_d = nc.dram_tensor("y", [NELEM, 4, TFULL], F32, kind="ExternalOutput")
    if decoder_only:
        xd_d = nc.dram_tensor("xd_in", [NELEM, 16, TENC], F32,
                              kind="ExternalInput")
    else:
        x_d = nc.dram_tensor("x", [NELEM, 4, TFULL], F32, kind="ExternalInput")
        dr_d = nc.dram_tensor("dist_rhs", [17, K_CAND], F32,
                              kind="ExternalInput")
        xdi_d = nc.dram_tensor("xdi", [K_CAND, 17], F32, kind="ExternalInput")
        triu_d = nc.dram_tensor("triu", [K_CAND, K_CAND], F32,
                                kind="ExternalInput")
        idx_d = nc.dram_tensor("idx", [NELEM * TENC], F32,
                               kind="ExternalOutput")
        msum_d = nc.dram_tensor("msum", [1, NELEM], F32, kind="ExternalOutput")
        x2s_d = nc.dram_tensor("x2sum", [128, 1], F32, kind="ExternalOutput")
        x2m_d = nc.dram_tensor("x2max", [128, 1], F32, kind="ExternalOutput")
        xe_d = nc.dram_tensor("xe_dump", [16, NELEM * TENC], F32,
                              kind="ExternalOutput")

    # ---- persistent SBUF ----
    wsb = nc.alloc_sbuf_tensor("wsb", [128, NW, 32], F32)
    bsb = nc.alloc_sbuf_tensor("bsb", [128, NB], F32)

    bufs = {}
    widths = {"A": 4096, "B": 2048, "C": 1024, "D": 512}
    counts = {"A": 3, "B": 4, "C": 4, "D": 4}
    for cls, T in widths.items():
        for i in range(counts[cls]):
            name = f"{cls}{i}"
            bufs[name] = nc.alloc_sbuf_tensor(name, [128, T + 2 * PAD], F32)
    bufs["XE"] = nc.alloc_sbuf_tensor("XE", [128, TENC], F32)
    bufs["XD"] = nc.alloc_sbuf_tensor("XD", [128, TENC + 2 * PAD], F32)

    if not decoder_only:
        flat16 = nc.alloc_sbuf_tensor("flat16", [17, NELEM * TENC], F32)
        drhs = nc.alloc_sbuf_tensor("drhs", [17, K_CAND], F32)
        xdi_s = nc.alloc_sbuf_tensor("xdi_s", [K_CAND, 17], F32)
        triu_s = nc.alloc_sbuf_tensor("triu_s", [K_CAND, K_CAND], F32)
        dnegT = nc.alloc_sbuf_tensor("dnegT", [K_CAND, 2, TENC], F32)
        mbc = nc.alloc_sbuf_tensor("mbc", [K_CAND, 2, TENC], F32)
        oh = nc.alloc_sbuf_tensor("oh", [K_CAND, 2, TENC + 2], F32)
        msc = nc.alloc_sbuf_tensor("msc", [1, 2, TENC], F32)
        mstat = nc.alloc_sbuf_tensor("mstat", [1, NELEM], F32)
        x2t = nc.alloc_sbuf_tensor("x2t", [128, TENC], F32)
        x2s = nc.alloc_sbuf_tensor("x2s", [128, 1], F32)
        x2m = nc.alloc_sbuf_tensor("x2m", [128, 1], F32)

    def interior(buf, start, count, step=1):
        """AP over all 128 partitions, columns [PAD+start : ...] step `step`."""
        t = bufs[buf]
        return bass.AP(tensor=t, offset=PAD + start,
                       ap=[[t.shape[-1], 128], [step, count]])

    def rows(buf, p0, np_, start, count, step=1, pad=PAD):
        t = bufs[buf]
        return bass.AP(tensor=t, offset=p0 * t.shape[-1] + pad + start,
                       ap=[[t.shape[-1], np_], [step, count]])

    with ExitStack() as ctx:
        tc = ctx.enter_context(tile.TileContext(nc))
        pool = ctx.enter_context(tc.tile_pool(name="ps", bufs=4, space="PSUM"))

        # ---------------- one-time init ----------------
        # zero whole input buffer (only 4 of every 16 partitions get data; the
        # rest must be 0.0 -- NaN garbage would poison 0-weight matmul taps)
        nc.gpsimd.memset(bufs["A0"][:, :], 0.0)
        nc.vector.memset(bufs["XD"][:, :], 0.0)
        if not decoder_only:
            nc.gpsimd.memset(oh[:, :, :], 0.0)
            # rows 0..15 are overwritten by the XE repack DMAs; row 16 stays 1.0
            nc.vector.memset(flat16[:, :], 1.0)
        # zero pad strips of all other activation buffers
        for name, t in bufs.items():
            if name in ("A0", "XD", "XE"):
                continue
            w = t.shape[-1]
            eng = nc.vector if (hash(name) & 1) else nc.gpsimd
            eng.memset(t[:, 0:PAD], 0.0)
            eng.memset(t[:, w - PAD:w], 0.0)

        # ---------------- param loads ----------------
        for r in range(4):
            nc.sync.dma_start(
                out=wsb[32 * r:32 * r + 32, :, :],
                in_=bass.AP(tensor=w_d, offset=0,
                            ap=[[32, 32], [32 * 32, NW], [1, 32]]))
        nc.sync.dma_start(out=bsb[:, :], in_=b_d.ap())
        if not decoder_only:
            nc.sync.dma_start(out=drhs[:, :], in_=dr_d.ap())
            nc.sync.dma_start(out=xdi_s[:, :], in_=xdi_d.ap())
            nc.sync.dma_start(out=triu_s[:, :], in_=triu_d.ap())
            # input x (host pre-transposed to (8, 4, 4096)): rows 16b..16b+3
            for b in range(NELEM):
                nc.sync.dma_start(
                    out=rows("A0", 16 * b, 4, 0, TFULL),
                    in_=bass.AP(tensor=x_d, offset=b * 4 * TFULL,
                                ap=[[TFULL, 4], [1, TFULL]]))
        else:
            for b in range(NELEM):
                nc.sync.dma_start(
                    out=rows("XD", 16 * b, 16, 0, TENC),
                    in_=bass.AP(tensor=xd_d, offset=b * 16 * TENC,
                                ap=[[TENC, 16], [1, TENC]]))

        # ---------------- helpers ----------------
        def emit_taps(ps_out_cols, taps, first, last):
            """taps: list of (wslot, rhs_ap_fn(g))."""
            for ti, (ws, rhs_fn) in enumerate(taps):
                st = first and ti == 0
                sp = last and ti == len(taps) - 1
                for g in range(4):
                    nc.tensor.matmul(
                        out=ps_out_cols(g),
                        lhsT=wsb[32 * g:32 * g + 32, ws, :],
                        rhs=rhs_fn(g),
                        start=st, stop=sp,
                        # the sim's psum group-check isn't partition-aware;
                        # the 4 concurrent 32-partition tile groups are
                        # independent accumulation groups on disjoint rows
                        skip_group_check=(g > 0),
                        tile_position=(32 * g, 32 * g))

        def bias_ap(nbi):
            return bsb[:, nbi:nbi + 1]

        def src_ap(sname, g, start, count, step=1):
            t = bufs[sname]
            base = 0 if sname == "XE" else PAD
            return bass.AP(tensor=t, offset=32 * g * t.shape[-1] + base + start,
                           ap=[[t.shape[-1], 32], [step, count]])

        def evict(mode, ps_ap, sname_dst, c0, cw, nbi, res_src=None, step=1):
            dst = interior(sname_dst, c0, cw, step) if sname_dst != "XE" else \
                bass.AP(tensor=bufs["XE"], offset=c0,
                        ap=[[TENC, 128], [1, cw]])
            if mode == "relu":
                nc.scalar.activation(out=dst, in_=ps_ap, func=ACTF.Relu,
                                     bias=bias_ap(nbi), scale=1.0)
            elif mode == "plain":
                nc.vector.tensor_scalar(out=dst, in0=ps_ap,
                                        scalar1=bias_ap(nbi), scalar2=None,
                                        op0=ALU.add)
            elif mode == "res":
                nc.vector.scalar_tensor_tensor(
                    out=dst, in0=ps_ap, scalar=bias_ap(nbi),
                    in1=interior(res_src, c0, cw), op0=ALU.add, op1=ALU.add)
            else:
                raise ValueError(mode)

        # ---------------- conv layers ----------------
        def emit_spec(s):
            kind = s["kind"]
            if kind == "relu":
                T = s["T"]
                nc.gpsimd.tensor_scalar(
                    out=interior(s["dst"], 0, T), in0=interior(s["src"], 0, T),
                    scalar1=0.0, scalar2=None, op0=ALU.max)
                return
            T = s["T"]
            nch = T // CHUNK if T >= CHUNK else 1
            cw = min(T, CHUNK)
            for ci in range(nch):
                ps = pool.tile([128, CHUNK], F32, tag="ps")

                def pcols(g, lo=0, n=None):
                    n = cw if n is None else n
                    return ps[32 * g:32 * g + 32, lo:lo + n]

                if kind == "conv3":
                    d = s["d"]
                    taps = [(s["w0"] + k,
                             (lambda k_: lambda g: src_ap(
                                 s["src"], g, ci * CHUNK + (k_ - 1) * d, cw))(k))
                            for k in range(3)]
                    emit_taps(lambda g: pcols(g), taps, True, True)
                elif kind == "conv1":
                    taps = [(s["w0"],
                             lambda g: src_ap(s["src"], g, ci * CHUNK, cw))]
                    emit_taps(lambda g: pcols(g), taps, True, True)
                elif kind == "conv4s2":
                    taps = [(s["w0"] + k,
                             (lambda k_: lambda g: src_ap(
                                 s["src"], g, 2 * ci * CHUNK + k_ - 1, cw,
                                 step=2))(k))
                            for k in range(4)]
                    emit_taps(lambda g: pcols(g), taps, True, True)
                elif kind == "upconv":
                    # output chunk [ci*512, ci*512+512), s = 256*ci + [0,256)
                    h = cw // 2
                    e_taps = [(s["w0"] + 0,
                               lambda g: src_ap(s["src"], g, ci * h - 1, h)),
                              (s["w0"] + 1,
                               lambda g: src_ap(s["src"], g, ci * h, h))]
                    o_taps = [(s["w0"] + 2,
                               lambda g: src_ap(s["src"], g, ci * h, h)),
                              (s["w0"] + 3,
                               lambda g: src_ap(s["src"], g, ci * h + 1, h))]
                    emit_taps(lambda g: pcols(g, 0, h), e_taps, True, False)
                    emit_taps(lambda g: pcols(g, h, h), o_taps, False, True)
                    evict("plain", ps[:, 0:h], s["dst"], ci * CHUNK, h, s["nb"],
                          step=2)
                    # odd phase -> columns ci*512+1, +3, ...
                    t = bufs[s["dst"]]
                    dst_odd = bass.AP(tensor=t,
                                      offset=PAD + ci * CHUNK + 1,
                                      ap=[[t.shape[-1], 128], [2, h]])
                    nc.vector.tensor_scalar(out=dst_odd, in0=ps[:, h:2 * h],
                                            scalar1=bias_ap(s["nb"]),
                                            scalar2=None, op0=ALU.add)
                    continue
                else:
                    raise ValueError(kind)
                evict(s["evict"], ps[:, 0:cw], s["dst"], ci * CHUNK, cw,
                      s["nb"], res_src=s.get("res"))

        # ---------------- VQ ----------------
        def emit_vq():
            # repack encoder output (128, 512) -> flat16 (16, 4096) via DMA;
            # row 16 of flat16 is constant 1.0 (folds -|c|^2 into the matmul)
            for b in range(NELEM):
                nc.sync.dma_start(
                    out=bass.AP(tensor=flat16, offset=TENC * b,
                                ap=[[NELEM * TENC, 16], [1, TENC]]),
                    in_=bass.AP(tensor=bufs["XE"],
                                offset=16 * b * TENC,
                                ap=[[TENC, 16], [1, TENC]]))
            # dump x_enc for the host-side fallback path
            nc.sync.dma_start(out=xe_d.ap(),
                              in_=bass.AP(tensor=flat16, offset=0,
                                          ap=[[NELEM * TENC, 16],
                                              [1, NELEM * TENC]]))
            # token stats for commit loss + validity bound
            nc.vector.tensor_tensor(out=x2t[:, :], in0=bufs["XE"][:, :],
                                    in1=bufs["XE"][:, :], op=ALU.mult)
            nc.vector.tensor_reduce(out=x2s[:, :], in_=x2t[:, :],
                                    axis=mybir.AxisListType.X, op=ALU.add)
            nc.vector.tensor_reduce(out=x2m[:, :], in_=x2t[:, :],
                                    axis=mybir.AxisListType.X, op=ALU.max)
            nc.sync.dma_start(out=x2s_d.ap(), in_=x2s[:, :])
            nc.sync.dma_start(out=x2m_d.ap(), in_=x2m[:, :])

            for b in range(NELEM):
                db = b % 2
                # dnegT[j, t] = 2 x_t . c_j - |c_j|^2   (codes on partitions)
                pv = pool.tile([128, CHUNK], F32, tag="ps")
                nc.tensor.matmul(
                    out=pv[0:K_CAND, :],
                    lhsT=drhs[:, :],
                    rhs=flat16[0:17, TENC * b:TENC * (b + 1)],
                    start=True, stop=True)
                nc.vector.tensor_copy(out=dnegT[:, db, :], in_=pv[0:K_CAND, :])
                # per-token max, broadcast to all 64 partitions
                nc.gpsimd.partition_all_reduce(
                    out_ap=mbc[:, db, :], in_ap=dnegT[:, db, :],
                    channels=K_CAND, reduce_op=bass_isa.ReduceOp.max)
                # commit-loss stat: sum_t max_t  (accumulated on ACT)
                nc.scalar.activation(out=msc[:, db, :], in_=mbc[0:1, db, :],
                                     func=ACTF.Copy,
                                     accum_out=mstat[:, b:b + 1])
                # raw one-hot of the argmax (ties -> multiple ones)
                nc.vector.tensor_tensor(out=oh[:, db, 1:TENC + 1],
                                        in0=dnegT[:, db, :],
                                        in1=mbc[:, db, :], op=ALU.is_equal)
                # exclusive first-match one-hot via prefix-count == 1
                pc = pool.tile([128, CHUNK], F32, tag="ps")
                nc.tensor.matmul(out=pc[0:K_CAND, :], lhsT=triu_s[:, :],
                                 rhs=oh[:, db, 1:TENC + 1],
                                 start=True, stop=True)
                nc.vector.scalar_tensor_tensor(
                    out=oh[:, db, 1:TENC + 1], in0=pc[0:K_CAND, :], scalar=1.0,
                    in1=oh[:, db, 1:TENC + 1], op0=ALU.is_equal, op1=ALU.mult)
                # x_d (rows 0..15) and winner slot index (row 16)
                px = pool.tile([128, CHUNK], F32, tag="ps")
                nc.tensor.matmul(out=px[0:17, :], lhsT=xdi_s[:, :],
                                 rhs=oh[:, db, 1:TENC + 1],
                                 start=True, stop=True)
                nc.scalar.activation(
                    out=bass.AP(tensor=flat16, offset=TENC * b,
                                ap=[[NELEM * TENC, 17], [1, TENC]]),
                    in_=px[0:17, :], func=ACTF.Copy)
            nc.sync.dma_start(out=idx_d.ap(), in_=flat16[16:17, :])
            nc.sync.dma_start(out=msum_d.ap(), in_=mstat[:, :])
            # repack x_d into decoder block layout
            for b in range(NELEM):
                nc.sync.dma_start(
                    out=rows("XD", 16 * b, 16, 0, TENC),
                    in_=bass.AP(tensor=flat16, offset=TENC * b,
                                ap=[[NELEM * TENC, 16], [1, TENC]]))

        # ---------------- main emission ----------------
        seen_vq = False
        for s in specs:
            if s["kind"] == "vq":
                seen_vq = True
                if not decoder_only:
                    emit_vq()
                continue
            if decoder_only and not seen_vq:
                continue
            emit_spec(s)

        # ---------------- output ----------------
        for b in range(NELEM):
            nc.sync.dma_start(
                out=bass.AP(tensor=y_d, offset=b * 4 * TFULL,
                            ap=[[TFULL, 4], [1, TFULL]]),
                in_=rows("A0", 16 * b, 4, 0, TFULL))

    nc.compile()
    return nc


def get_nc(decoder_only=False):
    key = ("dec" if decoder_only else "main")
    if key not in _NC_CACHE:
        specs, NW, NB = layer_specs()
        _NC_CACHE[key] = build_nc(NW, NB, decoder_only=decoder_only)
    return _NC_CACHE[key]


# --------------------------------------------------------------------------
# host entry point
# --------------------------------------------------------------------------

def _np_tree(params):
    if isinstance(params, dict):
        return {k: _np_tree(v) for k, v in params.items()}
    if isinstance(params, list):
        return [_np_tree(v) for v in params]
    return np.asarray(params, np.float32)


def kernel(x, params):
    x = np.ascontiguousarray(np.asarray(x, np.float32))
    params = _np_tree(params)
    wmats, bvecs = pack_conv_params(params)

    cb = np.ascontiguousarray(params["codebook"])  # (2048, 16)
    csq = (cb.astype(np.float64) ** 2).sum(1)
    csq32 = (cb ** 2).sum(1, dtype=np.float32)
    order = np.argsort(csq, kind="stable")
    cand = np.sort(order[:K_CAND])
    ccb = np.ascontiguousarray(cb[cand])
    ccsq = (ccb ** 2).sum(1, dtype=np.float32)
    dist_rhs = np.zeros((17, K_CAND), np.float32)
    dist_rhs[:16] = 2.0 * ccb.T
    dist_rhs[16] = -ccsq
    xdi = np.zeros((K_CAND, 17), np.float32)
    xdi[:, :16] = ccb
    xdi[:, 16] = np.arange(K_CAND, dtype=np.float32)
    triu = np.triu(np.ones((K_CAND, K_CAND), np.float32))

    xt = np.ascontiguousarray(np.transpose(x, (0, 2, 1)))  # (64, 4, 4096)

    nc = get_nc()
    base = {
        "wmats": wmats, "bvecs": bvecs,
        "dist_rhs": np.ascontiguousarray(dist_rhs),
        "xdi": np.ascontiguousarray(xdi),
        "triu": np.ascontiguousarray(triu),
    }
    in_maps = [dict(base, x=np.ascontiguousarray(xt[8 * c:8 * c + 8]))
               for c in range(8)]
    res = bass_utils.run_bass_kernel_spmd(nc, in_maps, core_ids=list(range(8)))
    cores = res.results

    y = np.concatenate([np.transpose(r["y"], (0, 2, 1)) for r in cores], 0)

    slots = np.concatenate([r["idx"] for r in cores]).astype(np.int64)
    msum = np.stack([r["msum"] for r in cores])           # (8, 1, 8)
    x2sum = np.stack([r["x2sum"] for r in cores])         # (8, 128, 1)
    x2max = np.stack([r["x2max"] for r in cores])

    # ---- validity of the candidate pruning (rigorous bound) ----
    # per-token ||x||^2 <= sum over the elem's 16 channels of max_t x^2
    r2_bound = max(
        float(x2max[c, 16 * b:16 * b + 16, 0].sum())
        for c in range(8) for b in range(NELEM))
    R = math.sqrt(max(r2_bound, 0.0))
    out_mask = np.ones(NCODE, bool)
    out_mask[cand] = False
    lout = float(np.min(csq[out_mask] - 2.0 * R * np.sqrt(csq[out_mask])))
    # max_t d*_t <= (R + sqrt(min cand csq))^2
    dstar_max = (R + math.sqrt(float(csq[cand].min()))) ** 2
    if not (dstar_max < lout):
        return _exact_fallback(x, params, wmats, bvecs, cb, csq32, cores)

    idx = cand[slots]                                     # original code ids
    counts = np.bincount(idx, minlength=NCODE).astype(np.float32)
    prob = counts / np.float32(idx.shape[0])
    perp = np.exp(-np.sum(prob * np.log(prob + np.float32(1e-7)),
                          dtype=np.float32))

    total = float(x2sum.sum(dtype=np.float64)) - float(
        msum.sum(dtype=np.float64))
    loss = np.float32(total / (idx.shape[0] * 16))

    return y, loss, np.float32(perp)


def _exact_fallback(x, params, wmats, bvecs, cb, csq32, cores):
    """Never taken for the reference input distribution: exact host argmin over
    the full codebook + device re-run of the decoder."""
    flat = np.concatenate([r["xe_dump"].T for r in cores], 0)  # (32768, 16)
    d = ((flat ** 2).sum(1, keepdims=True, dtype=np.float32)
         - 2.0 * (flat @ cb.T) + csq32[None, :])
    idx = d.argmin(1)
    x_dq = cb[idx]                                        # (32768, 16)
    x_st = flat + (x_dq - flat)
    counts = np.bincount(idx, minlength=NCODE).astype(np.float32)
    prob = counts / np.float32(idx.shape[0])
    perp = np.exp(-np.sum(prob * np.log(prob + np.float32(1e-7)),
                          dtype=np.float32))
    loss = np.float32(np.mean((flat - x_dq) ** 2, dtype=np.float64))

    nc = get_nc(decoder_only=True)
    xd = np.transpose(x_st.reshape(8, NELEM, TENC, 16), (0, 1, 3, 2))
    in_maps = [{"wmats": wmats, "bvecs": bvecs,
                "xd_in": np.ascontiguousarray(xd[c])} for c in range(8)]
    res = bass_utils.run_bass_kernel_spmd(nc, in_maps, core_ids=list(range(8)))
    y = np.concatenate(
        [np.transpose(r["y"], (0, 2, 1)) for r in res.results], 0)
    return y, loss, np.float32(perp)


# revision 33
# speedup vs baseline: 1.0521x; 1.0521x over previous
"""Trainium2 Bass kernel for nn_ActVQ (VQ-VAE: encoder -> vector-quantize -> decoder).

Contract: kernel(**inputs) takes the FULL inputs from reference.setup_inputs()
(x: (64, 4096, 4) f32 + nested params dict) and returns the FULL output tuple
(y: (64, 4096, 4) f32, commit_loss scalar, perplexity scalar), matching
reference.reference().

Sharding: pure data-parallel over batch N=64 -> 8 NeuronCores x 8 elements.
All parameters are replicated; outputs are gathered/combined on host.

Device layout: activations live in SBUF as (128, T) tiles with partition
p = 16*elem + channel (8 elems x 16 channels).  Each conv tap is one 128x128
matmul whose lhsT is a block-diagonal (8 x 16x16) channel-mixing matrix; the
K taps of a conv accumulate into PSUM, which is evicted by the scalar/vector
engines with bias add (+ ReLU / + residual) fused in.  Conv matmuls run in
float32r (reduced fp32, ~13-bit mantissa) which streams at full PE rate (4x
faster than fp32); the VQ math stays exact fp32.

VQ: distances argmin over the 2048-entry codebook is computed exactly over a
candidate subset of K=64 codes chosen on the host by smallest ||c||^2 (the
encoder output x_enc is tiny relative to codebook norms, so only the
smallest-norm codes can win).  The pruning is *validated at runtime* with a
rigorous bound from device outputs (max ||x_enc||); if the bound ever failed
(it cannot for the reference input distribution), the host falls back to an
exact argmin over all 2048 codes from the (also returned) encoder output and
re-runs the decoder on device.
"""

import math
from contextlib import ExitStack

import numpy as np

import concourse.bacc as bacc
import concourse.bass as bass
import concourse.bass_isa as bass_isa
import concourse.mybir as mybir
import concourse.tile as tile
from concourse import bass_utils

F32 = mybir.dt.float32
R32 = mybir.dt.float32r  # reduced-precision fp32 (~13-bit mantissa), 1 cyc/col
ALU = mybir.AluOpType
ACTF = mybir.ActivationFunctionType

PAD = 4          # halo columns on each side of activation buffers
K_CAND = 64      # candidate codebook entries (of 2048)
NCODE = 2048
NELEM = 8        # batch elements per core
TFULL = 4096
TENC = 512       # encoder output length per element
CHUNK = 512      # psum free-dim chunk

_NC_CACHE: dict = {}


# --------------------------------------------------------------------------
# static layer program (shared between host weight packer and device builder)
# --------------------------------------------------------------------------

def _cls_of(T):
    return {4096: "A", 2048: "B", 1024: "C", 512: "D"}[T]


def layer_specs():
    """Linear program of ops.  Weight-slot and bias-slot indices are assigned
    in order; the host packer follows the identical order."""
    specs = []
    wslot = [0]
    bslot = [0]

    def conv(kind, ntap, **kw):
        s = dict(kind=kind, w0=wslot[0], nb=bslot[0], **kw)
        wslot[0] += ntap
        bslot[0] += 1
        specs.append(s)

    def res_chain(cls, T, dils):
        xs = [cls + "0", cls + "3", cls + "0", cls + "3"]
        for j, d in enumerate(dils):
            specs.append(dict(kind="relu", T=T, src=xs[j], dst=cls + "1"))
            conv("conv3", 3, T=T, d=d, src=cls + "1", dst=cls + "2", evict="relu")
            conv("conv1", 1, T=T, src=cls + "2", dst=xs[j + 1], evict="res",
                 res=xs[j])
        return xs[3]

    # ---- encoder ----
    conv("conv3", 3, T=4096, d=1, src="A0", dst="A1", evict="relu")
    src = "A1"
    for T_in in (4096, 2048, 1024):
        cls = _cls_of(T_in // 2)
        conv("conv4s2", 4, T=T_in // 2, src=src, dst=cls + "0", evict="plain")
        src = res_chain(cls, T_in // 2, [1, 2, 4])
    conv("conv3", 3, T=512, d=1, src=src, dst="XE", evict="plain")
    n_enc_w = wslot[0]

    specs.append(dict(kind="vq"))

    # ---- decoder ----
    conv("conv3", 3, T=512, d=1, src="XD", dst="D0", evict="relu")
    up_dst = {512: "C0", 1024: "B0", 2048: "A0"}
    src = "D0"
    for T in (512, 1024, 2048):
        cls = _cls_of(T)
        last = res_chain(cls, T, [4, 2, 1])
        conv("upconv", 4, T=2 * T, src=last, dst=up_dst[T], evict="up")
        src = up_dst[T]
    conv("conv3", 3, T=4096, d=1, src="A0", dst="A1", evict="relu")
    conv("conv3", 3, T=4096, d=1, src="A1", dst="A0", evict="plain")

    return specs, wslot[0], bslot[0], n_enc_w


# --------------------------------------------------------------------------
# host-side parameter packing (runtime, numpy)
# --------------------------------------------------------------------------

def _diag2(w16):
    """w16: (out<=16, in<=16) conv-tap matrix -> (32, 32) lhsT block with the
    transposed tap on both 16x16 diagonal blocks (2 batch elems)."""
    m16 = np.zeros((16, 16), np.float32)
    m16[: w16.shape[0], : w16.shape[1]] = w16
    out = np.zeros((32, 32), np.float32)
    out[0:16, 0:16] = m16.T
    out[16:32, 16:32] = m16.T
    return out


def pack_conv_params(params):
    mats, biases = [], []

    def add_conv(w, b):
        w = np.asarray(w, np.float32)
        b = np.asarray(b, np.float32)
        for k in range(w.shape[2]):
            mats.append(_diag2(w[:, :, k]))
        biases.append(np.tile(np.pad(b, (0, 16 - b.shape[0])), NELEM))

    def add_upconv(w, b):
        w = np.asarray(w, np.float32)
        b = np.asarray(b, np.float32)
        for m in (w[:, :, 0], w[:, :, 1] + w[:, :, 2],
                  w[:, :, 0] + w[:, :, 1], w[:, :, 2]):
            mats.append(_diag2(m))
        biases.append(np.tile(np.pad(b, (0, 16 - b.shape[0])), NELEM))

    enc = params["enc"]
    add_conv(enc["in_w"], enc["in_b"])
    for blk in enc["down"]:
        add_conv(blk["w"], blk["b"])
        for rp in blk["res"]:
            add_conv(rp["w1"], rp["b1"])
            add_conv(rp["w2"], rp["b2"])
    add_conv(enc["out_w"], enc["out_b"])

    dec = params["dec"]
    add_conv(dec["in_w"], dec["in_b"])
    for blk in dec["up"]:
        for rp in blk["res"]:
            add_conv(rp["w1"], rp["b1"])
            add_conv(rp["w2"], rp["b2"])
        add_upconv(blk["w"], blk["b"])
    add_conv(dec["mid_w"], dec["mid_b"])
    add_conv(dec["out_w"], dec["out_b"])

    return (np.ascontiguousarray(np.stack(mats)),
            np.ascontiguousarray(np.stack(biases).astype(np.float32).T))


# --------------------------------------------------------------------------
# device kernel builder
# --------------------------------------------------------------------------

def build_nc(NW, NB, NWE, decoder_only=False):
    nc = bacc.Bacc("TRN2", target_bir_lowering=False, debug=False)

    specs, nw, nb, nwe = layer_specs()
    assert nw == NW and nb == NB and nwe == NWE

    # ---- DRAM I/O ----
    w_d = nc.dram_tensor("wmats", [NW, 32, 32], F32, kind="ExternalInput")
    b_d = nc.dram_tensor("bvecs", [128, NB], F32, kind="ExternalInput")
    y_d = nc.dram_tensor("y", [NELEM, 4, TFULL], F32, kind="ExternalOutput")
    if decoder_only:
        xd_d = nc.dram_tensor("xd_in", [NELEM, 16, TENC], F32,
                              kind="ExternalInput")
    else:
        x_d = nc.dram_tensor("x", [NELEM, 4, TFULL], F32, kind="ExternalInput")
        dr_d = nc.dram_tensor("dist_rhs", [17, K_CAND], F32,
                              kind="ExternalInput")
        xdi_d = nc.dram_tensor("xdi", [K_CAND, 17], F32, kind="ExternalInput")
        triu_d = nc.dram_tensor("triu", [K_CAND, K_CAND], F32,
                                kind="ExternalInput")
        idx_d = nc.dram_tensor("idx", [NELEM * TENC], F32,
                               kind="ExternalOutput")
        msum_d = nc.dram_tensor("msum", [1, NELEM], F32, kind="ExternalOutput")
        x2s_d = nc.dram_tensor("x2sum", [128, 1], F32, kind="ExternalOutput")
        x2m_d = nc.dram_tensor("x2max", [128, 1], F32, kind="ExternalOutput")
        xe_d = nc.dram_tensor("xe_dump", [16, NELEM * TENC], F32,
                              kind="ExternalOutput")

    # ---- persistent SBUF ----
    # conv weights/activations are float32r (reduced-precision fp32 streamed
    # at full PE rate); the VQ path (XE, flat16, onehot) stays exact fp32.
    # wsb holds full 128x128 block-diagonal tap matrices (f32r needs full
    # 128x128 PE mode); the diag blocks are cast-DMA'd in, the rest memset 0.
    wsb = nc.alloc_sbuf_tensor("wsb", [128, NW, 128], R32)
    bsb = nc.alloc_sbuf_tensor("bsb", [128, NB], F32)

    bufs = {}
    widths = {"A": 4096, "B": 2048, "C": 1024, "D": 512}
    counts = {"A": 2, "B": 4, "C": 4, "D": 4}
    for cls, T in widths.items():
        for i in range(counts[cls]):
            name = f"{cls}{i}"
            bufs[name] = nc.alloc_sbuf_tensor(name, [128, T + 2 * PAD], R32)
    bufs["XE"] = nc.alloc_sbuf_tensor("XE", [128, TENC], F32)
    bufs["XD"] = nc.alloc_sbuf_tensor("XD", [128, TENC + 2 * PAD], R32)

    if not decoder_only:
        flat16 = nc.alloc_sbuf_tensor("flat16", [17, NELEM * TENC], F32)
        drhs = nc.alloc_sbuf_tensor("drhs", [17, K_CAND], F32)
        xdi_s = nc.alloc_sbuf_tensor("xdi_s", [K_CAND, 17], F32)
        triu_s = nc.alloc_sbuf_tensor("triu_s", [K_CAND, K_CAND], F32)
        dnegT = nc.alloc_sbuf_tensor("dnegT", [K_CAND, 2, TENC], F32)
        mbc = nc.alloc_sbuf_tensor("mbc", [K_CAND, 2, TENC], F32)
        oh = nc.alloc_sbuf_tensor("oh", [K_CAND, 2, TENC + 2], F32)
        msc = nc.alloc_sbuf_tensor("msc", [1, 2, TENC], F32)
        mstat = nc.alloc_sbuf_tensor("mstat", [1, NELEM], F32)
        x2t = nc.alloc_sbuf_tensor("x2t", [128, TENC], F32)
        x2s = nc.alloc_sbuf_tensor("x2s", [128, 1], F32)
        x2m = nc.alloc_sbuf_tensor("x2m", [128, 1], F32)

    def interior(buf, start, count, step=1):
        t = bufs[buf]
        return bass.AP(tensor=t, offset=PAD + start,
                       ap=[[t.shape[-1], 128], [step, count]])

    def rows(buf, p0, np_, start, count, step=1, pad=PAD):
        t = bufs[buf]
        return bass.AP(tensor=t, offset=p0 * t.shape[-1] + pad + start,
                       ap=[[t.shape[-1], np_], [step, count]])

    with ExitStack() as ctx:
        tc = ctx.enter_context(tile.TileContext(nc))
        pool = ctx.enter_context(tc.tile_pool(name="ps", bufs=4, space="PSUM"))

        # ---------------- weights: zero + cast-DMA the diagonal blocks ----
        # split enc/dec so encoder weights are ready quickly
        for lo, hi in ((0, NWE), (NWE, NW)):
            n = hi - lo
            nc.vector.memset(
                bass.AP(tensor=wsb, offset=lo * 128,
                        ap=[[NW * 128, 128], [1, n * 128]]).bitcast(F32), 0.0)
            for r in range(4):
                nc.gpsimd.dma_start(
                    out=bass.AP(tensor=wsb,
                                offset=32 * r * NW * 128 + lo * 128 + 32 * r,
                                ap=[[NW * 128, 32], [128, n], [1, 32]]),
                    in_=bass.AP(tensor=w_d, offset=lo * 1024,
                                ap=[[32, 32], [1024, n], [1, 32]]))
        nc.sync.dma_start(out=bsb[:, :], in_=b_d.ap())

        # ---------------- one-time init ----------------
        # zero whole input buffer (only 4 of every 16 partitions get data;
        # NaN garbage would poison 0-weight matmul taps)
        nc.gpsimd.memset(bufs["A0"][:, :].bitcast(F32), 0.0)
        nc.vector.memset(bufs["XD"][:, :].bitcast(F32), 0.0)
        if not decoder_only:
            nc.gpsimd.memset(oh[:, :, :], 0.0)
            # rows 0..15 are overwritten by the XE repack DMAs; row 16 = 1.0
            nc.vector.memset(flat16[:, :], 1.0)
        # zero pad strips of all other activation buffers
        for name, t in bufs.items():
            if name in ("A0", "XD", "XE"):
                continue
            w = t.shape[-1]
            eng = nc.vector if (hash(name) & 1) else nc.gpsimd
            eng.memset(t[:, 0:PAD].bitcast(F32), 0.0)
            eng.memset(t[:, w - PAD:w].bitcast(F32), 0.0)

        # ---------------- inputs ----------------
        if not decoder_only:
            nc.sync.dma_start(out=drhs[:, :], in_=dr_d.ap())
            nc.sync.dma_start(out=xdi_s[:, :], in_=xdi_d.ap())
            nc.sync.dma_start(out=triu_s[:, :], in_=triu_d.ap())
            # input x (host pre-transposed to (8, 4, 4096)): rows 16b..16b+3
            # (gpsimd casting DMA rounds f32 -> f32r)
            for b in range(NELEM):
                nc.gpsimd.dma_start(
                    out=rows("A0", 16 * b, 4, 0, TFULL),
                    in_=bass.AP(tensor=x_d, offset=b * 4 * TFULL,
                                ap=[[TFULL, 4], [1, TFULL]]))
        else:
            for b in range(NELEM):
                nc.gpsimd.dma_start(
                    out=rows("XD", 16 * b, 16, 0, TENC),
                    in_=bass.AP(tensor=xd_d, offset=b * 16 * TENC,
                                ap=[[TENC, 16], [1, TENC]]))

        # ---------------- helpers ----------------
        def emit_taps(ps, cols, taps, first, last):
            """taps: list of (wslot, rhs_ap)."""
            for ti, (ws, rhs) in enumerate(taps):
                nc.tensor.matmul(
                    out=ps[:, cols[0]:cols[1]],
                    lhsT=wsb[:, ws, :],
                    rhs=rhs,
                    start=(first and ti == 0),
                    stop=(last and ti == len(taps) - 1))

        def bias_ap(nbi):
            return bsb[:, nbi:nbi + 1]

        def src_ap(sname, start, count, step=1):
            t = bufs[sname]
            base = 0 if sname == "XE" else PAD
            return bass.AP(tensor=t, offset=base + start,
                           ap=[[t.shape[-1], 128], [step, count]])

        def evict(mode, ps_ap, sname_dst, c0, cw, nbi, res_src=None, step=1):
            dst = interior(sname_dst, c0, cw, step) if sname_dst != "XE" else \
                bass.AP(tensor=bufs["XE"], offset=c0,
                        ap=[[TENC, 128], [1, cw]])
            if mode == "relu":
                nc.scalar.activation(out=dst, in_=ps_ap, func=ACTF.Relu,
                                     bias=bias_ap(nbi), scale=1.0)
            elif mode == "plain":
                nc.vector.tensor_scalar(out=dst, in0=ps_ap,
                                        scalar1=bias_ap(nbi), scalar2=None,
                                        op0=ALU.add)
            elif mode == "res":
                nc.vector.scalar_tensor_tensor(
                    out=dst, in0=ps_ap, scalar=bias_ap(nbi),
                    in1=interior(res_src, c0, cw), op0=ALU.add, op1=ALU.add)
            else:
                raise ValueError(mode)

        # ---------------- conv layers ----------------
        def emit_spec(s):
            kind = s["kind"]
            if kind == "relu":
                T = s["T"]
                nc.gpsimd.tensor_scalar(
                    out=interior(s["dst"], 0, T), in0=interior(s["src"], 0, T),
                    scalar1=0.0, scalar2=None, op0=ALU.max)
                return
            T = s["T"]
            nch = max(1, T // CHUNK)
            cw = min(T, CHUNK)
            for ci in range(nch):
                ps = pool.tile([128, CHUNK], F32, tag="ps", name="cps")

                if kind == "conv3":
                    d = s["d"]
                    taps = [(s["w0"] + k,
                             src_ap(s["src"], ci * CHUNK + (k - 1) * d, cw))
                            for k in range(3)]
                    emit_taps(ps, (0, cw), taps, True, True)
                elif kind == "conv1":
                    taps = [(s["w0"], src_ap(s["src"], ci * CHUNK, cw))]
                    emit_taps(ps, (0, cw), taps, True, True)
                elif kind == "conv4s2":
                    taps = [(s["w0"] + k,
                             src_ap(s["src"], 2 * ci * CHUNK + k - 1, cw,
                                    step=2))
                            for k in range(4)]
                    emit_taps(ps, (0, cw), taps, True, True)
                elif kind == "upconv":
                    h = cw // 2
                    e_taps = [(s["w0"] + 0, src_ap(s["src"], ci * h - 1, h)),
                              (s["w0"] + 1, src_ap(s["src"], ci * h, h))]
                    o_taps = [(s["w0"] + 2, src_ap(s["src"], ci * h, h)),
                              (s["w0"] + 3, src_ap(s["src"], ci * h + 1, h))]
                    emit_taps(ps, (0, h), e_taps, True, False)
                    emit_taps(ps, (h, 2 * h), o_taps, False, True)
                    evict("plain", ps[:, 0:h], s["dst"], ci * CHUNK, h,
                          s["nb"], step=2)
                    t = bufs[s["dst"]]
                    dst_odd = bass.AP(tensor=t, offset=PAD + ci * CHUNK + 1,
                                      ap=[[t.shape[-1], 128], [2, h]])
                    nc.vector.tensor_scalar(out=dst_odd, in0=ps[:, h:2 * h],
                                            scalar1=bias_ap(s["nb"]),
                                            scalar2=None, op0=ALU.add)
                    continue
                else:
                    raise ValueError(kind)
                evict(s["evict"], ps[:, 0:cw], s["dst"], ci * CHUNK, cw,
                      s["nb"], res_src=s.get("res"))

        # ---------------- VQ ----------------
        def emit_vq():
            # repack encoder output (128, 512) -> flat16 (16, 4096) via DMA;
            # row 16 of flat16 is constant 1.0 (folds -|c|^2 into the matmul)
            for b in range(NELEM):
                nc.sync.dma_start(
                    out=bass.AP(tensor=flat16, offset=TENC * b,
                                ap=[[NELEM * TENC, 16], [1, TENC]]),
                    in_=bass.AP(tensor=bufs["XE"],
                                offset=16 * b * TENC,
                                ap=[[TENC, 16], [1, TENC]]))
            # dump x_enc for the host-side fallback path
            nc.sync.dma_start(out=xe_d.ap(),
                              in_=bass.AP(tensor=flat16, offset=0,
                                          ap=[[NELEM * TENC, 16],
                                              [1, NELEM * TENC]]))
            # token stats for commit loss + validity bound
            nc.vector.tensor_tensor(out=x2t[:, :], in0=bufs["XE"][:, :],
                                    in1=bufs["XE"][:, :], op=ALU.mult)
            nc.vector.tensor_reduce(out=x2s[:, :], in_=x2t[:, :],
                                    axis=mybir.AxisListType.X, op=ALU.add)
            nc.vector.tensor_reduce(out=x2m[:, :], in_=x2t[:, :],
                                    axis=mybir.AxisListType.X, op=ALU.max)
            nc.sync.dma_start(out=x2s_d.ap(), in_=x2s[:, :])
            nc.sync.dma_start(out=x2m_d.ap(), in_=x2m[:, :])

            for b in range(NELEM):
                db = b % 2
                # dnegT[j, t] = 2 x_t . c_j - |c_j|^2   (codes on partitions)
                pv = pool.tile([128, CHUNK], F32, tag="ps", name="pv")
                nc.tensor.matmul(
                    out=pv[0:K_CAND, :],
                    lhsT=drhs[:, :],
                    rhs=flat16[0:17, TENC * b:TENC * (b + 1)],
                    start=True, stop=True)
                nc.vector.tensor_copy(out=dnegT[:, db, :], in_=pv[0:K_CAND, :])
                # per-token max over codes, broadcast to all 64 partitions
                nc.gpsimd.partition_all_reduce(
                    out_ap=mbc[:, db, :], in_ap=dnegT[:, db, :],
                    channels=K_CAND, reduce_op=bass_isa.ReduceOp.max)
                # commit-loss stat: sum_t m_t  (free ACT accumulator)
                nc.scalar.activation(out=msc[:, db, :], in_=mbc[0:1, db, :],
                                     func=ACTF.Copy,
                                     accum_out=mstat[:, b:b + 1])
                # raw one-hot of the argmax (exact ties -> multiple ones)
                nc.vector.tensor_tensor(out=oh[:, db, 1:TENC + 1],
                                        in0=dnegT[:, db, :],
                                        in1=mbc[:, db, :], op=ALU.is_equal)
                # exclusive first-match one-hot via prefix-count == 1
                pc = pool.tile([128, CHUNK], F32, tag="ps", name="pc")
                nc.tensor.matmul(out=pc[0:K_CAND, :], lhsT=triu_s[:, :],
                                 rhs=oh[:, db, 1:TENC + 1],
                                 start=True, stop=True)
                nc.vector.scalar_tensor_tensor(
                    out=oh[:, db, 1:TENC + 1], in0=pc[0:K_CAND, :], scalar=1.0,
                    in1=oh[:, db, 1:TENC + 1], op0=ALU.is_equal, op1=ALU.mult)
                # x_d (rows 0..15) and winner slot index (row 16)
                px = pool.tile([128, CHUNK], F32, tag="ps", name="px")
                nc.tensor.matmul(out=px[0:17, :], lhsT=xdi_s[:, :],
                                 rhs=oh[:, db, 1:TENC + 1],
                                 start=True, stop=True)
                nc.scalar.activation(
                    out=bass.AP(tensor=flat16, offset=TENC * b,
                                ap=[[NELEM * TENC, 17], [1, TENC]]),
                    in_=px[0:17, :], func=ACTF.Copy)
            nc.sync.dma_start(out=idx_d.ap(), in_=flat16[16:17, :])
            nc.sync.dma_start(out=msum_d.ap(), in_=mstat[:, :])
            # repack x_d into decoder block layout (casting DMA rounds to f32r)
            for b in range(NELEM):
                nc.gpsimd.dma_start(
                    out=rows("XD", 16 * b, 16, 0, TENC),
                    in_=bass.AP(tensor=flat16, offset=TENC * b,
                                ap=[[NELEM * TENC, 16], [1, TENC]]))

        # ---------------- main emission ----------------
        seen_vq = False
        for s in specs:
            if s["kind"] == "vq":
                seen_vq = True
                if not decoder_only:
                    emit_vq()
                continue
            if decoder_only and not seen_vq:
                continue
            emit_spec(s)

        # ---------------- output ----------------
        for b in range(NELEM):
            nc.sync.dma_start(
                out=bass.AP(tensor=y_d, offset=b * 4 * TFULL,
                            ap=[[TFULL, 4], [1, TFULL]]),
                in_=rows("A0", 16 * b, 4, 0, TFULL).bitcast(F32))

    nc.compile()
    return nc


def get_nc(decoder_only=False):
    key = ("dec" if decoder_only else "main")
    if key not in _NC_CACHE:
        specs, NW, NB, NWE = layer_specs()
        _NC_CACHE[key] = build_nc(NW, NB, NWE, decoder_only=decoder_only)
    return _NC_CACHE[key]


# --------------------------------------------------------------------------
# host entry point
# --------------------------------------------------------------------------

def _np_tree(params):
    if isinstance(params, dict):
        return {k: _np_tree(v) for k, v in params.items()}
    if isinstance(params, list):
        return [_np_tree(v) for v in params]
    return np.asarray(params, np.float32)


def kernel(x, params):
    x = np.ascontiguousarray(np.asarray(x, np.float32))
    params = _np_tree(params)
    wmats, bvecs = pack_conv_params(params)

    cb = np.ascontiguousarray(params["codebook"])  # (2048, 16)
    csq = (cb.astype(np.float64) ** 2).sum(1)
    csq32 = (cb ** 2).sum(1, dtype=np.float32)
    order = np.argsort(csq, kind="stable")
    cand = np.sort(order[:K_CAND])
    ccb = np.ascontiguousarray(cb[cand])
    ccsq = (ccb ** 2).sum(1, dtype=np.float32)
    dist_rhs = np.zeros((17, K_CAND), np.float32)
    dist_rhs[:16] = 2.0 * ccb.T
    dist_rhs[16] = -ccsq
    xdi = np.zeros((K_CAND, 17), np.float32)
    xdi[:, :16] = ccb
    xdi[:, 16] = np.arange(K_CAND, dtype=np.float32)
    triu = np.triu(np.ones((K_CAND, K_CAND), np.float32))

    xt = np.ascontiguousarray(np.transpose(x, (0, 2, 1)))  # (64, 4, 4096)

    nc = get_nc()
    base = {
        "wmats": wmats, "bvecs": bvecs,
        "dist_rhs": np.ascontiguousarray(dist_rhs),
        "xdi": np.ascontiguousarray(xdi),
        "triu": np.ascontiguousarray(triu),
    }
    in_maps = [dict(base, x=np.ascontiguousarray(xt[8 * c:8 * c + 8]))
               for c in range(8)]
    res = bass_utils.run_bass_kernel_spmd(nc, in_maps, core_ids=list(range(8)))
    cores = res.results

    y = np.concatenate([np.transpose(r["y"], (0, 2, 1)) for r in cores], 0)

    slots = np.concatenate([r["idx"] for r in cores]).astype(np.int64)
    msum = np.stack([r["msum"] for r in cores])           # (8, 1, 8)
    x2sum = np.stack([r["x2sum"] for r in cores])         # (8, 128, 1)
    x2max = np.stack([r["x2max"] for r in cores])

    # ---- validity of the candidate pruning (rigorous bound) ----
    # per-token ||x||^2 <= sum over the elem's 16 channels of max_t x^2
    r2_bound = max(
        float(x2max[c, 16 * b:16 * b + 16, 0].sum())
        for c in range(8) for b in range(NELEM))
    R = math.sqrt(max(r2_bound, 0.0))
    out_mask = np.ones(NCODE, bool)
    out_mask[cand] = False
    lout = float(np.min(csq[out_mask] - 2.0 * R * np.sqrt(csq[out_mask])))
    # max_t d*_t <= (R + sqrt(min cand csq))^2
    dstar_max = (R + math.sqrt(float(csq[cand].min()))) ** 2
    if not (dstar_max < lout):
        return _exact_fallback(x, params, wmats, bvecs, cb, csq32, cores)

    idx = cand[slots]                                     # original code ids
    counts = np.bincount(idx, minlength=NCODE).astype(np.float32)
    prob = counts / np.float32(idx.shape[0])
    perp = np.exp(-np.sum(prob * np.log(prob + np.float32(1e-7)),
                          dtype=np.float32))

    total = float(x2sum.sum(dtype=np.float64)) - float(
        msum.sum(dtype=np.float64))
    loss = np.float32(total / (idx.shape[0] * 16))

    return y, loss, np.float32(perp)


def _exact_fallback(x, params, wmats, bvecs, cb, csq32, cores):
    """Never taken for the reference input distribution: exact host argmin over
    the full codebook + device re-run of the decoder."""
    flat = np.concatenate([r["xe_dump"].T for r in cores], 0)  # (32768, 16)
    d = ((flat ** 2).sum(1, keepdims=True, dtype=np.float32)
         - 2.0 * (flat @ cb.T) + csq32[None, :])
    idx = d.argmin(1)
    x_dq = cb[idx]                                        # (32768, 16)
    x_st = flat + (x_dq - flat)
    counts = np.bincount(idx, minlength=NCODE).astype(np.float32)
    prob = counts / np.float32(idx.shape[0])
    perp = np.exp(-np.sum(prob * np.log(prob + np.float32(1e-7)),
                          dtype=np.float32))
    loss = np.float32(np.mean((flat - x_dq) ** 2, dtype=np.float64))

    nc = get_nc(decoder_only=True)
    xd = np.transpose(x_st.reshape(8, NELEM, TENC, 16), (0, 1, 3, 2))
    in_maps = [{"wmats": wmats, "bvecs": bvecs,
                "xd_in": np.ascontiguousarray(xd[c])} for c in range(8)]
    res = bass_utils.run_bass_kernel_spmd(nc, in_maps, core_ids=list(range(8)))
    y = np.concatenate(
        [np.transpose(r["y"], (0, 2, 1)) for r in res.results], 0)
    return y, loss, np.float32(perp)
